# revision 33
# baseline (speedup 1.0000x reference)
"""Trainium2 Bass kernel for the FC-SNN (LIF hidden layer + LI readout).

Structure (per core, batch-sharded B=512 -> 64):
  host:   i1[t] is a spike-independent linear filter of x, so it is folded into
          a prefilter of x (exact reparameterization); layout/transpose/pad.
  device: S = (0.1*xfilt) @ w1T  (big matmul, time-parallel)
          v-scan over t (threshold + reset, the only sequential part)
          oc = z @ w_outT        (spike readout matmul)
          LI readout (vo/io scans) + max over t
  host:   gather [10,64] per core -> [512,10].

Modes:
  r1  1-pass fp32r fc1 (11-bit pre-round), f32 state, fp16 spikes/w2.
      rel err ~5.7e-3. DEFAULT.
  h1  fp16 everywhere: same speed as r1 (PE-bound) but rel err ~1.8e-2 —
      too close to the 2e-2 gate; kept for reference.
  bf3 3-pass split bf16 (near-exact, 3x slower; debugging).

Measured TRN2 laws (axon cores, repeat-loop differencing):
  - PE streams matmul columns at ~1.85 Gcol/s (f32r/fp16/bf16 alike,
    ap 256 or 512 alike, PSUM chain length irrelevant) => fc1 floor
    459k cols ~= 250us, fc_out 65k ~= 35us. Kernel ~295us = floor.
  - gpsimd (Pool) COMPUTE ops cost ~14us each (software dispatch): never
    compute there. gpsimd DMA triggering is the FAST path; sync(SP)
    dma_start costs ~12us per call: x DMAs must ride gpsimd.
  - Strided stationary (lhsT) halves the PE column rate: keep weight
    chunks contiguous; sg is m-major so fc_out's moving operand is
    contiguous too.
  - DVE: 2-src f32 ops are 1x (~1.1us per 128x1024); single-src
    tensor_scalar gets 2x; fp16 2-src gets 2x_1p (h1 scan uses that).
"""

import sys

if "/opt/trn_rl_repo" not in sys.path:
    sys.path.insert(0, "/opt/trn_rl_repo")

from contextlib import ExitStack

import numpy as np

# problem dims (hardcoded per contract)
T, B, C, Hh, Ww = 64, 512, 1, 28, 28
IN, HID, OUT = 784, 2048, 10
NCORES = 8
BL = B // NCORES            # 64 batch rows per core
TB = T * BL                 # 4096 matmul columns per core
KC = 7                      # contraction chunks: 784 padded to 896 = 7*128
MC = HID // 128             # 16 hidden chunks
TG = 4                      # time steps per pipeline group
NG = T // TG                # 16 groups
NCOL = TG * BL              # 256 columns per group
NVD = 6                     # vd state ring buffers

MODE = "r1"
TRACE = False

_CACHE = {}
LAST_RESULT = None


def _round_mant(a, mbits):
    """Round fp32 mantissa to mbits (round-to-nearest, matching fp32r pre-round)."""
    ai = np.ascontiguousarray(a, np.float32).view(np.uint32).astype(np.uint64)
    half = np.uint64(1) << np.uint64(22 - mbits)
    mask = np.uint64(0xFFFFFFFF) << np.uint64(23 - mbits)
    return ((ai + half) & mask).astype(np.uint32).view(np.float32)


def _to_bf16(a):
    import ml_dtypes
    return np.ascontiguousarray(a).astype(ml_dtypes.bfloat16)


def _layout_x(arr):
    """[TB, IN] -> [128, KC, TB] (pad IN to 896, p-major k-chunks)."""
    a = np.zeros((KC * 128, TB), arr.dtype)
    a[:IN, :] = arr.T
    return np.ascontiguousarray(a.reshape(KC, 128, TB).transpose(1, 0, 2))


def _layout_w(arr):
    """[IN, HID] -> [128, MC, KC*128] (m-major so per-m DMA is contiguous)."""
    a = np.zeros((KC * 128, HID), arr.dtype)
    a[:IN, :] = arr
    # [KC,128,HID] -> [128, MC, KC, 128q]
    b = a.reshape(KC, 128, MC, 128).transpose(1, 2, 0, 3)
    return np.ascontiguousarray(b.reshape(128, MC, KC * 128))


def _layout_w2(arr):
    """[HID, OUT] -> [128, MC*OUT]."""
    return np.ascontiguousarray(
        arr.reshape(MC, 128, OUT).transpose(1, 0, 2).reshape(128, MC * OUT)
    )


ENG_OVERRIDE = {}   # experiment knobs; defaults below are the tuned config


def _build_nc(mode, tg=TG, repeat=1, level=5):
    import concourse.bacc as bacc
    import concourse.mybir as mybir
    import concourse.tile as tile

    f32 = mybir.dt.float32
    f32r = mybir.dt.float32r
    bf16 = mybir.dt.bfloat16
    f16 = mybir.dt.float16
    Alu = mybir.AluOpType

    nc = bacc.Bacc("TRN2", debug=False)
    ncol = tg * BL
    ng = T // tg
    nvd = NVD if tg <= 4 else 4
    sgbufs = 2 if tg <= 4 else 1

    if mode == "h1":
        xdt = wdt = sdt = f16     # matmul input / state dtype
        w2dt = f16
        nxs, nws = 1, 1
        passes = [(0, 0)]
        z_on_pool = False
    elif mode == "r1":
        xdt = wdt = f32r
        sdt = f32
        w2dt = f16                # spikes are exact in f16; w2 f16 = 11-bit
        nxs, nws = 1, 1
        passes = [(0, 0)]
        z_on_pool = False
    else:
        xdt = wdt = bf16
        sdt = f32
        w2dt = f32r
        nxs, nws = 2, 2
        passes = [(0, 0), (0, 1), (1, 0)]
        z_on_pool = False
    sgdt = f16 if mode in ("h1", "r1") else f32r
    if ENG_OVERRIDE.get("z") == "pool":
        z_on_pool = True
    elif ENG_OVERRIDE.get("z") == "dve":
        z_on_pool = False

    xps = [
        nc.declare_dram_parameter(f"x{i}", [128, KC, TB], xdt, isOutput=False)
        for i in range(nxs)
    ]
    wps = [
        nc.declare_dram_parameter(f"w{i}", [128, MC, KC * 128], wdt, isOutput=False)
        for i in range(nws)
    ]
    w2ps = [
        nc.declare_dram_parameter("w20", [128, MC * OUT], w2dt, isOutput=False)
    ]
    vmax_p = nc.declare_dram_parameter("vmax", [OUT, BL], f32, isOutput=True)

    with tile.TileContext(nc) as tc, ExitStack() as ctx:
        const = ctx.enter_context(tc.tile_pool(name="const", bufs=1))
        xpool = ctx.enter_context(tc.tile_pool(
            name="x", bufs=ENG_OVERRIDE.get("xbufs", 3 if tg <= 4 else 2)))
        swpool = ctx.enter_context(tc.tile_pool(name="sw", bufs=2))
        sgpool = ctx.enter_context(tc.tile_pool(name="sg", bufs=sgbufs))
        ocpool = ctx.enter_context(tc.tile_pool(name="oc", bufs=2))
        psS = ctx.enter_context(tc.tile_pool(
            name="psS", bufs=ENG_OVERRIDE.get("psS", 5), space="PSUM"))
        psO = ctx.enter_context(tc.tile_pool(
            name="psO", bufs=ENG_OVERRIDE.get("psO", 2), space="PSUM"))
        psOt = ctx.enter_context(tc.tile_pool(name="psOt", bufs=1, space="PSUM"))

        # persistent tiles; w split per m-chunk so the first fc1 only waits
        # for w[m=0] + x[g=0] instead of the whole 7 MB load.
        w_eng = getattr(nc, ENG_OVERRIDE.get("w", "gpsimd"))
        x_eng = getattr(nc, ENG_OVERRIDE.get("x", "gpsimd"))

        def dma_x(g):
            tiles = []
            for si in range(nxs):
                xt = xpool.tile([128, KC * ncol], xdt, tag=f"x{si}", name=f"x{si}")
                x_eng.dma_start(
                    xt[:].rearrange("p (k c) -> p k c", k=KC),
                    xps[si][:, :, g * ncol:(g + 1) * ncol],
                )
                tiles.append(xt)
            return tiles

        # single-launch: put x[g=0] at the head of the DMA queue, ahead of
        # the one-time w stream (in repeat mode the loop hides w anyway).
        first_xts = dma_x(0) if repeat == 1 else None
        wts = []
        for i in range(nws):
            wm = []
            for m in range(MC):
                wt = const.tile(
                    [128, KC * 128], wdt, tag=f"w{i}m{m}", name=f"w{i}m{m}"
                )
                w_eng.dma_start(wt[:], wps[i][:, m, :])
                wm.append(wt)
            wts.append(wm)
        w2t = const.tile([128, MC * OUT], w2dt, tag="w20", name="w20")
        w_eng.dma_start(w2t[:], w2ps[0][:, :])
        vd = [const.tile([128, MC * BL], sdt, tag=f"vd{i}", name=f"vd{i}") for i in range(nvd)]
        vr = const.tile([128, MC * BL], sdt, tag="vr", name="vr")
        zb = const.tile([128, MC * BL], sdt, tag="zb", name="zb")
        js = const.tile([OUT, BL], f32, tag="js", name="js")
        vo = const.tile([OUT, BL], f32, tag="vo", name="vo")
        vmax_t = const.tile([OUT, BL], f32, tag="vmax", name="vmax")

        def fc1(g, xts):
            sw = swpool.tile([128, tg * MC * BL], sdt, tag="swin", name="swin")
            for m in range(MC):
                ps = psS.tile([128, ncol], f32, tag="psS", name="psS")
                idx, last = 0, len(passes) * KC - 1
                for (xi, wi) in passes:
                    wap = wts[wi][m][:].rearrange("p (k q) -> p k q", k=KC)
                    xap = xts[xi][:].rearrange("p (k c) -> p k c", k=KC)
                    for k in range(KC):
                        lhsT = wap[:, k, :]
                        rhs = xap[:, k, :]
                        nc.tensor.matmul(
                            ps[:], lhsT, rhs, start=(idx == 0), stop=(idx == last)
                        )
                        idx += 1
                if level >= 1:
                    if ENG_OVERRIDE.get("swm", True):
                        dst = sw[:].rearrange(
                            "p (m t b) -> p m t b", m=MC, t=tg
                        )[:, m, :, :]
                        nc.scalar.copy(dst, ps[:])
                    else:
                        dst = sw[:].rearrange(
                            "p (t m b) -> p t m b", t=tg, m=MC
                        )[:, :, m, :]
                        nc.scalar.copy(
                            dst, ps[:].rearrange("p (t b) -> p t b", t=tg))
            return sw

        def scan_step(t, sw, sg, tloc):
            w = MC * BL
            a = vd[t % nvd]
            b = vd[(t + 1) % nvd]
            # sg is m-major (m, t, b) so fc_out's moving operand is contiguous
            sg_t = sg[:].rearrange(
                "p (m t b) -> p m t b", m=MC, t=tg)[:, :, tloc, :]
            if ENG_OVERRIDE.get("swm", True):
                sw_t = sw[:].rearrange(
                    "p (m t b) -> p m t b", m=MC, t=tg)[:, :, tloc, :]
            else:
                sw_t = sw[:, tloc * w:(tloc + 1) * w]
            if True:
                if mode == "h1":
                    # fp16 path: keep every op in a fast DVE perf mode.
                    # zb = 0.9*(a<=0.5)  (tensor_scalar: 4x mode)
                    # vr = zb*a          (tensor_tensor: 2x_1p)
                    # b  = vr + sw       (tensor_tensor: 2x_1p)
                    nc.vector.tensor_scalar(
                        zb[:], a[:], 0.5, 0.9, op0=Alu.is_le, op1=Alu.mult
                    )
                    if level >= 3:
                        nc.vector.tensor_scalar(
                            sg_t, a[:], 0.5, None, op0=Alu.is_gt
                        )
                    nc.vector.tensor_tensor(
                        vr[:], zb[:], a[:], op=Alu.mult
                    )
                    nc.vector.tensor_tensor(
                        b[:], vr[:], sw_t, op=Alu.add
                    )
                else:
                    nc.vector.scalar_tensor_tensor(
                        vr[:], a[:], 0.5, a[:], op0=Alu.is_le, op1=Alu.mult
                    )
                    if level >= 3:
                        zeng = nc.gpsimd if z_on_pool else nc.vector
                        zeng.tensor_scalar(
                            sg_t, a[:], 0.5, None, op0=Alu.is_gt
                        )
                    nc.vector.scalar_tensor_tensor(
                        b[:], vr[:], 0.9, sw_t,
                        op0=Alu.mult, op1=Alu.add,
                    )

        def scan(g, sw):
            sg = sgpool.tile([128, tg * MC * BL], sgdt, tag="sgn", name="sgn")
            if level < 2:
                return sg
            for tloc in range(tg):
                scan_step(g * tg + tloc, sw, sg, tloc)
            return sg

        def fc_out(g, sg):
            po = psO.tile([OUT, ncol], f32, tag="psO", name="psO")
            if level < 4:
                return ocpool.tile([OUT, ncol], f32, tag="ocs", name="ocs")
            sgap = sg[:].rearrange("p (m t b) -> p m t b", m=MC, t=tg)
            for m in range(MC):
                rhs = sgap[:, m, :, :]
                lhsT = w2t[:, m * OUT:(m + 1) * OUT]
                nc.tensor.matmul(
                    po[:], lhsT, rhs, start=(m == 0), stop=(m == MC - 1)
                )
            oc = ocpool.tile([OUT, ncol], f32, tag="ocs", name="ocs")
            nc.scalar.copy(oc[:], po[:])
            return oc

        def readout(g, oc):
            if level < 5:
                return
            for tloc in range(tg):
                nc.vector.scalar_tensor_tensor(
                    vo[:], vo[:], 0.9, js[:], op0=Alu.mult, op1=Alu.add
                )
                nc.vector.tensor_tensor(vmax_t[:], vmax_t[:], vo[:], op=Alu.max)
                nc.vector.scalar_tensor_tensor(
                    js[:], js[:], 0.8, oc[:, tloc * BL:(tloc + 1) * BL],
                    op0=Alu.mult, op1=Alu.add,
                )

        def fc_out_t(sg, tloc):
            """Single-timestep fc_out (64-col matmuls) for the drain tail."""
            if level < 4:
                return None
            po = psOt.tile([OUT, BL], f32, tag="psOt", name="psOt")
            sgap = sg[:].rearrange("p (m t b) -> p m t b", m=MC, t=tg)
            for m in range(MC):
                nc.tensor.matmul(
                    po[:], w2t[:, m * OUT:(m + 1) * OUT], sgap[:, m, tloc, :],
                    start=(m == 0), stop=(m == MC - 1)
                )
            oc = ocpool.tile([OUT, BL], f32, tag="oct", name="oct")
            nc.scalar.copy(oc[:], po[:])
            return oc

        def readout_t(oc):
            if level < 5 or oc is None:
                return
            nc.vector.scalar_tensor_tensor(
                vo[:], vo[:], 0.9, js[:], op0=Alu.mult, op1=Alu.add
            )
            nc.vector.tensor_tensor(vmax_t[:], vmax_t[:], vo[:], op=Alu.max)
            nc.vector.scalar_tensor_tensor(
                js[:], js[:], 0.8, oc[:, :], op0=Alu.mult, op1=Alu.add
            )

        def whole_body():
            # init state, then software-pipelined emission: fc1 leads the
            # scan by 1 group, the readout chain trails by 2 so no engine
            # head-of-line blocks another.
            nc.vector.memset(vd[0][:], 0.0)
            nc.vector.memset(js[:], 0.0)
            nc.vector.memset(vo[:], 0.0)
            nc.vector.memset(vmax_t[:], 0.0)
            xts = first_xts if first_xts is not None else dma_x(0)
            sws, sgs, ocs = {}, {}, {}
            sws[0] = fc1(0, xts)
            fcout_first = ENG_OVERRIDE.get("fcout_first", False)
            for g in range(1, ng):
                xts = dma_x(g)
                if fcout_first and g >= 2:
                    # fc_out(g-2) ahead of fc1(g) in the PE queue: if x(g) is
                    # late, the PE still has DMA-independent work first.
                    ocs[g - 2] = fc_out(g - 2, sgs.pop(g - 2))
                sws[g] = fc1(g, xts)
                sgs[g - 1] = scan(g - 1, sws.pop(g - 1))
                if g >= 2:
                    if not fcout_first:
                        ocs[g - 2] = fc_out(g - 2, sgs.pop(g - 2))
                    readout(g - 2, ocs.pop(g - 2))
            # drain: next-to-last group's fc_out runs on PE while the last
            # group's scan advances on DVE; the last group is then emitted
            # per-timestep so each fc_out_t follows its scan step immediately.
            ocs[ng - 2] = fc_out(ng - 2, sgs.pop(ng - 2))
            readout(ng - 2, ocs.pop(ng - 2))
            sw_last = sws.pop(ng - 1)
            sg_last = sgpool.tile([128, tg * MC * BL], sgdt, tag="sgn", name="sgn")
            if level >= 2:
                for tloc in range(tg):
                    scan_step((ng - 1) * tg + tloc, sw_last, sg_last, tloc)
                    readout_t(fc_out_t(sg_last, tloc))
            nc.sync.dma_start(vmax_p[:, :], vmax_t[:])

        if repeat > 1:
            with tc.For_i(0, repeat, 1):
                whole_body()
        else:
            whole_body()

    nc.compile()
    return nc


def _prep_inputs(x, w1, w_out, mode):
    x = np.ascontiguousarray(x, np.float32).reshape(T, B, IN)
    # i1[t] = 0.8*i1[t-1] + x[t] @ w1T  ==  prefilter(x)[t] @ w1T
    xf = np.empty_like(x)
    acc = np.zeros((B, IN), np.float32)
    e8 = np.float32(0.8)
    for t in range(T):
        acc = e8 * acc + x[t]
        xf[t] = acc
    xs = np.float32(0.1) * xf                       # S = xs @ w1T
    w1T = np.ascontiguousarray(w1, np.float32).T    # [IN, HID]

    if mode == "h1":
        xparts = [xs.astype(np.float16)]
        wparts = [_layout_w(w1T.astype(np.float16))]
    elif mode == "r1":
        xparts = [_round_mant(xs, 11)]
        wparts = [_layout_w(_round_mant(w1T, 11))]
    else:
        xh = _to_bf16(xs)
        xl = _to_bf16(xs - xh.astype(np.float32))
        xparts = [xh, xl]
        wh = _to_bf16(w1T)
        wl = _to_bf16(w1T - wh.astype(np.float32))
        wparts = [_layout_w(wh), _layout_w(wl)]

    w2 = np.float32(0.1) * np.ascontiguousarray(w_out, np.float32).T  # [HID,OUT]
    if mode in ("h1", "r1"):
        w2l = _layout_w2(w2.astype(np.float16))
    else:
        w2l = _layout_w2(_round_mant(w2, 11))

    common = {"w20": w2l}
    for i, wp in enumerate(wparts):
        common[f"w{i}"] = wp

    in_maps = []
    for c in range(NCORES):
        m = dict(common)
        for i, xp in enumerate(xparts):
            xc = xp[:, c * BL:(c + 1) * BL, :].reshape(TB, IN)
            m[f"x{i}"] = _layout_x(xc)
        in_maps.append(m)
    return in_maps


def kernel(x, w1, w_out):
    global LAST_RESULT
    from concourse.bass_utils import run_bass_kernel_spmd

    if MODE not in _CACHE:
        _CACHE[MODE] = _build_nc(MODE, tg=TG)
    nc = _CACHE[MODE]
    in_maps = _prep_inputs(np.asarray(x), np.asarray(w1), np.asarray(w_out), MODE)
    res = run_bass_kernel_spmd(nc, in_maps, list(range(NCORES)), trace=TRACE)
    LAST_RESULT = res
    out = np.empty((B, OUT), np.float32)
    for c in range(NCORES):
        out[c * BL:(c + 1) * BL, :] = np.asarray(res.results[c]["vmax"]).T
    return out


# revision 34
# speedup vs baseline: 1.0203x; 1.0203x over previous
"""Trainium2 Bass kernel for the FC-SNN (LIF hidden layer + LI readout).

Structure (per core, batch-sharded B=512 -> 64):
  host:   i1[t] is a spike-independent linear filter of x, so it is folded into
          a prefilter of x (exact reparameterization); layout/transpose/pad.
  device: S = (0.1*xfilt) @ w1T  (big matmul, time-parallel)
          v-scan over t (threshold + reset, the only sequential part)
          oc = z @ w_outT        (spike readout matmul)
          LI readout (vo/io scans) + max over t
  host:   gather [10,64] per core -> [512,10].

Modes:
  r1  1-pass fp32r fc1 (11-bit pre-round), f32 state, fp16 spikes/w2.
      rel err ~5.7e-3. DEFAULT.
  h1  fp16 everywhere: same speed as r1 (PE-bound) but rel err ~1.8e-2 —
      too close to the 2e-2 gate; kept for reference.
  bf3 3-pass split bf16 (near-exact, 3x slower; debugging).

Measured TRN2 laws (axon cores, repeat-loop differencing):
  - PE streams matmul columns at ~1.85 Gcol/s (f32r/fp16/bf16 alike,
    ap 256 or 512 alike, PSUM chain length irrelevant) => fc1 floor
    459k cols ~= 250us, fc_out 65k ~= 35us. Kernel ~295us = floor.
  - gpsimd (Pool) COMPUTE ops cost ~14us each (software dispatch): never
    compute there. gpsimd DMA triggering is the FAST path; sync(SP)
    dma_start costs ~12us per call: x DMAs must ride gpsimd.
  - Strided stationary (lhsT) halves the PE column rate: keep weight
    chunks contiguous; sg is m-major so fc_out's moving operand is
    contiguous too.
  - DVE: 2-src f32 ops are 1x (~1.1us per 128x1024); single-src
    tensor_scalar gets 2x; fp16 2-src gets 2x_1p (h1 scan uses that).
"""

import sys

if "/opt/trn_rl_repo" not in sys.path:
    sys.path.insert(0, "/opt/trn_rl_repo")

from contextlib import ExitStack

import numpy as np

# problem dims (hardcoded per contract)
T, B, C, Hh, Ww = 64, 512, 1, 28, 28
IN, HID, OUT = 784, 2048, 10
NCORES = 8
BL = B // NCORES            # 64 batch rows per core
TB = T * BL                 # 4096 matmul columns per core
KC = 7                      # contraction chunks: 784 padded to 896 = 7*128
MC = HID // 128             # 16 hidden chunks
TG = 4                      # time steps per pipeline group
NG = T // TG                # 16 groups
NCOL = TG * BL              # 256 columns per group
NVD = 6                     # vd state ring buffers

MODE = "r1"
TRACE = False

_CACHE = {}
LAST_RESULT = None


def _round_mant(a, mbits):
    """Round fp32 mantissa to mbits (round-to-nearest, matching fp32r pre-round)."""
    ai = np.ascontiguousarray(a, np.float32).view(np.uint32).astype(np.uint64)
    half = np.uint64(1) << np.uint64(22 - mbits)
    mask = np.uint64(0xFFFFFFFF) << np.uint64(23 - mbits)
    return ((ai + half) & mask).astype(np.uint32).view(np.float32)


def _to_bf16(a):
    import ml_dtypes
    return np.ascontiguousarray(a).astype(ml_dtypes.bfloat16)


def _layout_x(arr):
    """[TB, IN] -> [128, KC, TB] (pad IN to 896, p-major k-chunks)."""
    a = np.zeros((KC * 128, TB), arr.dtype)
    a[:IN, :] = arr.T
    return np.ascontiguousarray(a.reshape(KC, 128, TB).transpose(1, 0, 2))


def _layout_w(arr):
    """[IN, HID] -> [128, MC, KC*128] (m-major so per-m DMA is contiguous)."""
    a = np.zeros((KC * 128, HID), arr.dtype)
    a[:IN, :] = arr
    # [KC,128,HID] -> [128, MC, KC, 128q]
    b = a.reshape(KC, 128, MC, 128).transpose(1, 2, 0, 3)
    return np.ascontiguousarray(b.reshape(128, MC, KC * 128))


def _layout_w2(arr):
    """[HID, OUT] -> [128, MC*OUT]."""
    return np.ascontiguousarray(
        arr.reshape(MC, 128, OUT).transpose(1, 0, 2).reshape(128, MC * OUT)
    )


ENG_OVERRIDE = {}   # experiment knobs; defaults below are the tuned config


def _build_nc(mode, tg=TG, repeat=1, level=5):
    import concourse.bacc as bacc
    import concourse.mybir as mybir
    import concourse.tile as tile

    f32 = mybir.dt.float32
    f32r = mybir.dt.float32r
    bf16 = mybir.dt.bfloat16
    f16 = mybir.dt.float16
    Alu = mybir.AluOpType

    nc = bacc.Bacc("TRN2", debug=False)
    ncol = tg * BL
    ng = T // tg
    nvd = NVD if tg <= 4 else 4
    sgbufs = 2 if tg <= 4 else 1

    if mode == "h1":
        xdt = wdt = sdt = f16     # matmul input / state dtype
        w2dt = f16
        nxs, nws = 1, 1
        passes = [(0, 0)]
        z_on_pool = False
    elif mode == "r1":
        xdt = wdt = f32r
        sdt = f32
        w2dt = f16                # spikes are exact in f16; w2 f16 = 11-bit
        nxs, nws = 1, 1
        passes = [(0, 0)]
        z_on_pool = False
    else:
        xdt = wdt = bf16
        sdt = f32
        w2dt = f32r
        nxs, nws = 2, 2
        passes = [(0, 0), (0, 1), (1, 0)]
        z_on_pool = False
    sgdt = f16 if mode in ("h1", "r1") else f32r
    if ENG_OVERRIDE.get("z") == "pool":
        z_on_pool = True
    elif ENG_OVERRIDE.get("z") == "dve":
        z_on_pool = False

    xps = [
        nc.declare_dram_parameter(f"x{i}", [128, KC, TB], xdt, isOutput=False)
        for i in range(nxs)
    ]
    wps = [
        nc.declare_dram_parameter(f"w{i}", [128, MC, KC * 128], wdt, isOutput=False)
        for i in range(nws)
    ]
    w2ps = [
        nc.declare_dram_parameter("w20", [128, MC * OUT], w2dt, isOutput=False)
    ]
    vmax_p = nc.declare_dram_parameter("vmax", [OUT, BL], f32, isOutput=True)

    with tile.TileContext(nc) as tc, ExitStack() as ctx:
        const = ctx.enter_context(tc.tile_pool(name="const", bufs=1))
        xpool = ctx.enter_context(tc.tile_pool(
            name="x", bufs=ENG_OVERRIDE.get("xbufs", 3 if tg <= 4 else 2)))
        swpool = ctx.enter_context(tc.tile_pool(
            name="sw", bufs=ENG_OVERRIDE.get("swbufs", 2)))
        sgpool = ctx.enter_context(tc.tile_pool(
            name="sg", bufs=ENG_OVERRIDE.get("sgbufs", sgbufs)))
        ocpool = ctx.enter_context(tc.tile_pool(name="oc", bufs=2))
        psS = ctx.enter_context(tc.tile_pool(
            name="psS", bufs=ENG_OVERRIDE.get("psS", 5), space="PSUM"))
        psO = ctx.enter_context(tc.tile_pool(
            name="psO", bufs=ENG_OVERRIDE.get("psO", 2), space="PSUM"))
        psOt = ctx.enter_context(tc.tile_pool(name="psOt", bufs=1, space="PSUM"))

        # persistent tiles; w split per m-chunk so the first fc1 only waits
        # for w[m=0] + x[g=0] instead of the whole 7 MB load.
        w_eng = getattr(nc, ENG_OVERRIDE.get("w", "gpsimd"))
        x_eng = getattr(nc, ENG_OVERRIDE.get("x", "gpsimd"))

        def dma_x(g):
            tiles = []
            for si in range(nxs):
                xt = xpool.tile([128, KC * ncol], xdt, tag=f"x{si}", name=f"x{si}")
                x_eng.dma_start(
                    xt[:].rearrange("p (k c) -> p k c", k=KC),
                    xps[si][:, :, g * ncol:(g + 1) * ncol],
                )
                tiles.append(xt)
            return tiles

        # single-launch: put x[g=0] at the head of the DMA queue, ahead of
        # the one-time w stream (in repeat mode the loop hides w anyway).
        first_xts = dma_x(0) if repeat == 1 else None
        wts = []
        for i in range(nws):
            wm = []
            for m in range(MC):
                wt = const.tile(
                    [128, KC * 128], wdt, tag=f"w{i}m{m}", name=f"w{i}m{m}"
                )
                w_eng.dma_start(wt[:], wps[i][:, m, :])
                wm.append(wt)
            wts.append(wm)
        w2t = const.tile([128, MC * OUT], w2dt, tag="w20", name="w20")
        w_eng.dma_start(w2t[:], w2ps[0][:, :])
        vd = [const.tile([128, MC * BL], sdt, tag=f"vd{i}", name=f"vd{i}") for i in range(nvd)]
        vr = const.tile([128, MC * BL], sdt, tag="vr", name="vr")
        zb = const.tile([128, MC * BL], sdt, tag="zb", name="zb")
        js = const.tile([OUT, BL], f32, tag="js", name="js")
        vo = const.tile([OUT, BL], f32, tag="vo", name="vo")
        vmax_t = const.tile([OUT, BL], f32, tag="vmax", name="vmax")

        def fc1(g, xts):
            sw = swpool.tile([128, tg * MC * BL], sdt, tag="swin", name="swin")
            for m in range(MC):
                ps = psS.tile([128, ncol], f32, tag="psS", name="psS")
                idx, last = 0, len(passes) * KC - 1
                for (xi, wi) in passes:
                    wap = wts[wi][m][:].rearrange("p (k q) -> p k q", k=KC)
                    xap = xts[xi][:].rearrange("p (k c) -> p k c", k=KC)
                    for k in range(KC):
                        lhsT = wap[:, k, :]
                        rhs = xap[:, k, :]
                        nc.tensor.matmul(
                            ps[:], lhsT, rhs, start=(idx == 0), stop=(idx == last)
                        )
                        idx += 1
                if level >= 1:
                    if ENG_OVERRIDE.get("swm", True):
                        dst = sw[:].rearrange(
                            "p (m t b) -> p m t b", m=MC, t=tg
                        )[:, m, :, :]
                        nc.scalar.copy(dst, ps[:])
                    else:
                        dst = sw[:].rearrange(
                            "p (t m b) -> p t m b", t=tg, m=MC
                        )[:, :, m, :]
                        nc.scalar.copy(
                            dst, ps[:].rearrange("p (t b) -> p t b", t=tg))
            return sw

        def scan_step(t, sw, sg, tloc):
            w = MC * BL
            a = vd[t % nvd]
            b = vd[(t + 1) % nvd]
            # sg is m-major (m, t, b) so fc_out's moving operand is contiguous
            sg_t = sg[:].rearrange(
                "p (m t b) -> p m t b", m=MC, t=tg)[:, :, tloc, :]
            if ENG_OVERRIDE.get("swm", True):
                sw_t = sw[:].rearrange(
                    "p (m t b) -> p m t b", m=MC, t=tg)[:, :, tloc, :]
            else:
                sw_t = sw[:, tloc * w:(tloc + 1) * w]
            if True:
                if mode == "h1":
                    # fp16 path: keep every op in a fast DVE perf mode.
                    # zb = 0.9*(a<=0.5)  (tensor_scalar: 4x mode)
                    # vr = zb*a          (tensor_tensor: 2x_1p)
                    # b  = vr + sw       (tensor_tensor: 2x_1p)
                    nc.vector.tensor_scalar(
                        zb[:], a[:], 0.5, 0.9, op0=Alu.is_le, op1=Alu.mult
                    )
                    if level >= 3:
                        nc.vector.tensor_scalar(
                            sg_t, a[:], 0.5, None, op0=Alu.is_gt
                        )
                    nc.vector.tensor_tensor(
                        vr[:], zb[:], a[:], op=Alu.mult
                    )
                    nc.vector.tensor_tensor(
                        b[:], vr[:], sw_t, op=Alu.add
                    )
                else:
                    nc.vector.scalar_tensor_tensor(
                        vr[:], a[:], 0.5, a[:], op0=Alu.is_le, op1=Alu.mult
                    )
                    if level >= 3:
                        zeng = nc.gpsimd if z_on_pool else nc.vector
                        zeng.tensor_scalar(
                            sg_t, a[:], 0.5, None, op0=Alu.is_gt
                        )
                    nc.vector.scalar_tensor_tensor(
                        b[:], vr[:], 0.9, sw_t,
                        op0=Alu.mult, op1=Alu.add,
                    )

        def scan(g, sw):
            sg = sgpool.tile([128, tg * MC * BL], sgdt, tag="sgn", name="sgn")
            if level < 2:
                return sg
            for tloc in range(tg):
                scan_step(g * tg + tloc, sw, sg, tloc)
            return sg

        def fc_out(g, sg):
            po = psO.tile([OUT, ncol], f32, tag="psO", name="psO")
            if level < 4:
                return ocpool.tile([OUT, ncol], f32, tag="ocs", name="ocs")
            sgap = sg[:].rearrange("p (m t b) -> p m t b", m=MC, t=tg)
            for m in range(MC):
                rhs = sgap[:, m, :, :]
                lhsT = w2t[:, m * OUT:(m + 1) * OUT]
                nc.tensor.matmul(
                    po[:], lhsT, rhs, start=(m == 0), stop=(m == MC - 1)
                )
            oc = ocpool.tile([OUT, ncol], f32, tag="ocs", name="ocs")
            nc.scalar.copy(oc[:], po[:])
            return oc

        def readout(g, oc):
            if level < 5:
                return
            for tloc in range(tg):
                nc.vector.scalar_tensor_tensor(
                    vo[:], vo[:], 0.9, js[:], op0=Alu.mult, op1=Alu.add
                )
                nc.vector.tensor_tensor(vmax_t[:], vmax_t[:], vo[:], op=Alu.max)
                nc.vector.scalar_tensor_tensor(
                    js[:], js[:], 0.8, oc[:, tloc * BL:(tloc + 1) * BL],
                    op0=Alu.mult, op1=Alu.add,
                )

        def fc_out_t(sg, tloc):
            """Single-timestep fc_out (64-col matmuls) for the drain tail."""
            if level < 4:
                return None
            po = psOt.tile([OUT, BL], f32, tag="psOt", name="psOt")
            sgap = sg[:].rearrange("p (m t b) -> p m t b", m=MC, t=tg)
            for m in range(MC):
                nc.tensor.matmul(
                    po[:], w2t[:, m * OUT:(m + 1) * OUT], sgap[:, m, tloc, :],
                    start=(m == 0), stop=(m == MC - 1)
                )
            oc = ocpool.tile([OUT, BL], f32, tag="oct", name="oct")
            nc.scalar.copy(oc[:], po[:])
            return oc

        def readout_t(oc):
            if level < 5 or oc is None:
                return
            nc.vector.scalar_tensor_tensor(
                vo[:], vo[:], 0.9, js[:], op0=Alu.mult, op1=Alu.add
            )
            nc.vector.tensor_tensor(vmax_t[:], vmax_t[:], vo[:], op=Alu.max)
            nc.vector.scalar_tensor_tensor(
                js[:], js[:], 0.8, oc[:, :], op0=Alu.mult, op1=Alu.add
            )

        def whole_body():
            # init state, then software-pipelined emission: fc1 leads the
            # scan by 1 group, the readout chain trails by 2 so no engine
            # head-of-line blocks another.
            nc.vector.memset(vd[0][:], 0.0)
            nc.vector.memset(js[:], 0.0)
            nc.vector.memset(vo[:], 0.0)
            nc.vector.memset(vmax_t[:], 0.0)
            xts = first_xts if first_xts is not None else dma_x(0)
            sws, sgs, ocs = {}, {}, {}
            sws[0] = fc1(0, xts)
            fcout_first = ENG_OVERRIDE.get("fcout_first", False)
            for g in range(1, ng):
                xts = dma_x(g)
                if fcout_first and g >= 2:
                    # fc_out(g-2) ahead of fc1(g) in the PE queue: if x(g) is
                    # late, the PE still has DMA-independent work first.
                    ocs[g - 2] = fc_out(g - 2, sgs.pop(g - 2))
                sws[g] = fc1(g, xts)
                sgs[g - 1] = scan(g - 1, sws.pop(g - 1))
                if g >= 2:
                    if not fcout_first:
                        ocs[g - 2] = fc_out(g - 2, sgs.pop(g - 2))
                    readout(g - 2, ocs.pop(g - 2))
            # drain: next-to-last group's fc_out runs on PE while the last
            # group's scan advances on DVE; the last group is then emitted
            # per-timestep so each fc_out_t follows its scan step immediately.
            ocs[ng - 2] = fc_out(ng - 2, sgs.pop(ng - 2))
            readout(ng - 2, ocs.pop(ng - 2))
            sw_last = sws.pop(ng - 1)
            sg_last = sgpool.tile([128, tg * MC * BL], sgdt, tag="sgn", name="sgn")
            if level >= 2:
                for tloc in range(tg):
                    scan_step((ng - 1) * tg + tloc, sw_last, sg_last, tloc)
                    readout_t(fc_out_t(sg_last, tloc))
            nc.sync.dma_start(vmax_p[:, :], vmax_t[:])

        if repeat > 1:
            with tc.For_i(0, repeat, 1):
                whole_body()
        else:
            whole_body()

    nc.compile()
    return nc


def _prep_inputs(x, w1, w_out, mode):
    x = np.ascontiguousarray(x, np.float32).reshape(T, B, IN)
    # i1[t] = 0.8*i1[t-1] + x[t] @ w1T  ==  prefilter(x)[t] @ w1T
    xf = np.empty_like(x)
    acc = np.zeros((B, IN), np.float32)
    e8 = np.float32(0.8)
    for t in range(T):
        acc = e8 * acc + x[t]
        xf[t] = acc
    xs = np.float32(0.1) * xf                       # S = xs @ w1T
    w1T = np.ascontiguousarray(w1, np.float32).T    # [IN, HID]

    if mode == "h1":
        xparts = [xs.astype(np.float16)]
        wparts = [_layout_w(w1T.astype(np.float16))]
    elif mode == "r1":
        xparts = [_round_mant(xs, 11)]
        wparts = [_layout_w(_round_mant(w1T, 11))]
    else:
        xh = _to_bf16(xs)
        xl = _to_bf16(xs - xh.astype(np.float32))
        xparts = [xh, xl]
        wh = _to_bf16(w1T)
        wl = _to_bf16(w1T - wh.astype(np.float32))
        wparts = [_layout_w(wh), _layout_w(wl)]

    w2 = np.float32(0.1) * np.ascontiguousarray(w_out, np.float32).T  # [HID,OUT]
    if mode in ("h1", "r1"):
        w2l = _layout_w2(w2.astype(np.float16))
    else:
        w2l = _layout_w2(_round_mant(w2, 11))

    common = {"w20": w2l}
    for i, wp in enumerate(wparts):
        common[f"w{i}"] = wp

    in_maps = []
    for c in range(NCORES):
        m = dict(common)
        for i, xp in enumerate(xparts):
            xc = xp[:, c * BL:(c + 1) * BL, :].reshape(TB, IN)
            m[f"x{i}"] = _layout_x(xc)
        in_maps.append(m)
    return in_maps


def kernel(x, w1, w_out):
    global LAST_RESULT
    from concourse.bass_utils import run_bass_kernel_spmd

    if MODE not in _CACHE:
        _CACHE[MODE] = _build_nc(MODE, tg=TG)
    nc = _CACHE[MODE]
    in_maps = _prep_inputs(np.asarray(x), np.asarray(w1), np.asarray(w_out), MODE)
    res = run_bass_kernel_spmd(nc, in_maps, list(range(NCORES)), trace=TRACE)
    LAST_RESULT = res
    out = np.empty((B, OUT), np.float32)
    for c in range(NCORES):
        out[c * BL:(c + 1) * BL, :] = np.asarray(res.results[c]["vmax"]).T
    return out
